# revision 60
# baseline (speedup 1.0000x reference)
"""Multi-head attention (B=2, S=2048, D=1024, H=16) on 8 trn2 NeuronCores.

Sharding: data-parallel over batch (cores 0-3 -> b=0, cores 4-7 -> b=1),
tensor-parallel over heads within each batch group (4 heads per core).
Each core:
  - projects q/k/v with its 256-column slice of Wq/Wk/Wv (heads h0..h0+3),
  - computes scores/softmax/PV for its 4 heads (emitting the probs slice),
  - computes a partial out-projection with its 256-row slice of Wo.
Host passes activations pre-transposed to [d_model, s] (layout prep, like
the weight slices), gathers probs slices, and sums the partial outputs.

Projection order v, k, q: the last projection before attention is q, whose
first s-chunk unblocks head 0's first panel, so the probs DMA stream starts
while later q chunks are still loading.

All matmuls run as float32r (full PE rate at N>=256, ~1e-4 rel err).
PSUM pools are shared across stages via common tags (no pool barriers).
"""

import numpy as np

import concourse.bass as bass
import concourse.mybir as mybir
import concourse.tile as tile
from concourse import bacc
from concourse.bass_utils import run_bass_kernel_spmd
from concourse.masks import make_identity

B = 2
S = 2048
DM = 1024
H = 16
DK = 64
NCORES = 8
HPC = 4            # heads per core
DPC = HPC * DK     # 256: d_out slice per core
NPAIR = HPC // 2   # 2 head pairs per core
F32 = mybir.dt.float32
F32R = mybir.dt.float32r

PANEL = 512        # q rows per phase-2 panel
NPANEL = S // PANEL
QSUB = 128         # q rows per scores matmul
NQSUB = PANEL // QSUB
KC = 512           # k columns per scores matmul
NKC = S // KC
SC = 512           # s columns per phase-1 chunk
NSC = S // SC
NKB = DM // 128    # d_model k-chunks for projections


def build_nc():
    nc = bacc.Bacc("TRN2", target_bir_lowering=False)

    xqt = nc.dram_tensor("xqt", [DM, S], F32R, kind="ExternalInput")
    xkt = nc.dram_tensor("xkt", [DM, S], F32R, kind="ExternalInput")
    xvt = nc.dram_tensor("xvt", [DM, S], F32R, kind="ExternalInput")
    wq = nc.dram_tensor("wq", [DM, DPC], F32R, kind="ExternalInput")
    wk = nc.dram_tensor("wk", [DM, DPC], F32R, kind="ExternalInput")
    wv = nc.dram_tensor("wv", [DM, DPC], F32R, kind="ExternalInput")
    wo = nc.dram_tensor("wo", [DPC, DM], F32R, kind="ExternalInput")
    bq = nc.dram_tensor("bq", [DPC], F32, kind="ExternalInput")
    bk = nc.dram_tensor("bk", [DPC], F32, kind="ExternalInput")
    bv = nc.dram_tensor("bv", [DPC], F32, kind="ExternalInput")

    probs_out = nc.dram_tensor("probs", [HPC, S, S], F32R, kind="ExternalOutput")
    y_out = nc.dram_tensor("y", [S, DM], F32, kind="ExternalOutput")

    with tile.TileContext(nc) as tc:
        with (
            tc.tile_pool(name="persist", bufs=1) as persist,
            tc.tile_pool(name="ps_big", bufs=2, space="PSUM") as ps_big,
            tc.tile_pool(name="ps_mid", bufs=2, space="PSUM") as ps_mid,
            tc.tile_pool(name="ps_sml", bufs=2, space="PSUM") as ps_sml,
        ):
            ident_f = persist.tile([128, 128], F32)
            make_identity(nc, ident_f)
            ident = persist.tile([128, 128], F32R)
            nc.vector.tensor_copy(ident, ident_f)

            qT = persist.tile([128, NPAIR, S], F32R)
            kT = persist.tile([128, NPAIR, S], F32R)
            vN = persist.tile([128, S // 128, DPC], F32R)
            attnT = persist.tile([128, NPAIR, S], F32R)
            wo_sb = persist.tile([128, NPAIR, DM], F32R)

            # ---------------- Phase 1: projections (v, k, q) ----------------
            with (
                tc.tile_pool(name="ph1w", bufs=1) as ph1w,
                tc.tile_pool(name="ph1", bufs=3) as ph1,
            ):
                xTv0 = ph1.tile([128, NKB, SC], F32R, tag="xT")
                nc.sync.dma_start(
                    out=xTv0,
                    in_=xvt[:, 0:SC].rearrange("(kb p) s -> p kb s", p=128),
                )
                wv_sb = ph1w.tile([128, NKB, DPC], F32R)
                wk_sb = ph1w.tile([128, NKB, DPC], F32R)
                wq_sb = ph1w.tile([128, NKB, DPC], F32R)
                nc.sync.dma_start(
                    out=wv_sb, in_=wv[:].rearrange("(kc p) n -> p kc n", p=128)
                )
                nc.sync.dma_start(
                    out=wk_sb, in_=wk[:].rearrange("(kc p) n -> p kc n", p=128)
                )
                nc.sync.dma_start(
                    out=wq_sb, in_=wq[:].rearrange("(kc p) n -> p kc n", p=128)
                )
                bq_sb = ph1w.tile([128, 2], F32)
                bk_sb = ph1w.tile([128, 2], F32)
                nc.sync.dma_start(out=bq_sb, in_=bq[:].rearrange("(m p) -> p m", p=128))
                nc.sync.dma_start(out=bk_sb, in_=bk[:].rearrange("(m p) -> p m", p=128))
                bv_sb = ph1w.tile([128, DPC], F32)
                bv_ap = bv[:]
                nc.sync.dma_start(
                    out=bv_sb,
                    in_=bass.AP(
                        tensor=bv_ap.tensor, offset=bv_ap.offset,
                        ap=[[0, 128], [1, DPC]],
                    ),
                )

                for which, xinT in (("v", xvt), ("k", xkt), ("q", xqt)):
                    for sc in range(NSC):
                        if which == "v" and sc == 0:
                            xTt = xTv0
                        else:
                            xTt = ph1.tile([128, NKB, SC], F32R, tag="xT")
                            nc.sync.dma_start(
                                out=xTt,
                                in_=xinT[:, sc * SC:(sc + 1) * SC].rearrange(
                                    "(kb p) s -> p kb s", p=128
                                ),
                            )
                        if which == "v":
                            for ss in range(SC // 128):
                                psv = ps_sml.tile([128, DPC], F32, tag="s")
                                for kb in range(NKB):
                                    nc.tensor.matmul(
                                        psv,
                                        xTt[:, kb, ss * 128:(ss + 1) * 128],
                                        wv_sb[:, kb, :],
                                        start=(kb == 0),
                                        stop=(kb == NKB - 1),
                                    )
                                nc.vector.tensor_add(
                                    vN[:, sc * (SC // 128) + ss, :], psv, bv_sb
                                )
                        else:
                            dst = qT if which == "q" else kT
                            bias = bq_sb if which == "q" else bk_sb
                            w_sb = wq_sb if which == "q" else wk_sb
                            for m in range(NPAIR):
                                psq = ps_mid.tile([128, SC], F32, tag="m")
                                for kb in range(NKB):
                                    nc.tensor.matmul(
                                        psq,
                                        w_sb[:, kb, m * 128:(m + 1) * 128],
                                        xTt[:, kb, :],
                                        start=(kb == 0),
                                        stop=(kb == NKB - 1),
                                    )
                                nc.vector.tensor_scalar_add(
                                    dst[:, m, sc * SC:(sc + 1) * SC],
                                    psq,
                                    bias[:, m:m + 1],
                                )

            # ---------------- Phase 2: attention per head ----------------
            with tc.tile_pool(name="ph2", bufs=1) as ph2:
                nc.sync.dma_start(
                    out=wo_sb, in_=wo[:].rearrange("(m p) n -> p m n", p=128)
                )
                for h in range(HPC):
                    p, hh = h // 2, h % 2
                    dlo = 64 * hh
                    for pan in range(NPANEL):
                        for half in range(NQSUB // 2):
                            pT = ph2.tile([128, S // 128, PANEL // 2], F32R,
                                          tag="pT", bufs=2)
                            eh = ph2.tile([128, 2, S], F32R, tag="exp", bufs=4)
                            for q2 in range(2):
                                qs = half * 2 + q2
                                q0 = pan * PANEL + qs * QSUB
                                den = ph2.tile([128, 2], F32, tag="den", bufs=6)
                                for kh in range(2):
                                    ps = ps_big.tile([128, 2 * KC], F32, tag="sc")
                                    for k2 in range(2):
                                        kc = 2 * kh + k2
                                        nc.tensor.matmul(
                                            ps[:, k2 * KC:(k2 + 1) * KC],
                                            qT[dlo:dlo + 64, p, q0:q0 + QSUB],
                                            kT[dlo:dlo + 64, p,
                                               kc * KC:(kc + 1) * KC],
                                            start=True, stop=True,
                                        )
                                    nc.scalar.activation(
                                        out=eh[:, q2, kh * 1024:(kh + 1) * 1024],
                                        in_=ps,
                                        func=mybir.ActivationFunctionType.Exp,
                                        scale=0.125,
                                        accum_out=den[:, kh:kh + 1],
                                    )
                                rcp = ph2.tile([128, 1], F32, tag="rcp", bufs=6)
                                nc.vector.reduce_sum(
                                    out=rcp, in_=den, axis=mybir.AxisListType.X
                                )
                                nc.vector.reciprocal(out=rcp, in_=rcp)
                                nc.vector.tensor_scalar_mul(
                                    eh[:, q2, :], eh[:, q2, :], rcp
                                )
                                for kbg in range(4):
                                    pst2 = ps_mid.tile([128, 4, 128], F32R, tag="m")
                                    for k4 in range(4):
                                        kb = kbg * 4 + k4
                                        nc.tensor.transpose(
                                            pst2[:, k4, :],
                                            eh[:, q2, kb * 128:(kb + 1) * 128],
                                            ident,
                                        )
                                    dst = pT[:, kbg * 4:(kbg + 1) * 4,
                                             q2 * 128:(q2 + 1) * 128]
                                    if (q2 * 4 + kbg) % 8 < 5:
                                        nc.vector.tensor_copy(dst, pst2)
                                    else:
                                        nc.scalar.copy(dst, pst2)
                            q0 = pan * PANEL + half * 2 * QSUB
                            nc.sync.dma_start(
                                out=probs_out[h, q0:q0 + 2 * QSUB, :].rearrange(
                                    "(q2 pp) k -> pp q2 k", pp=128
                                ),
                                in_=eh,
                            )
                            pspv = ps_sml.tile([64, PANEL // 2], F32, tag="s")
                            lo = h * DK
                            for kb in range(S // 128):
                                nc.tensor.matmul(
                                    pspv,
                                    vN[:, kb, lo:lo + DK],
                                    pT[:, kb, :],
                                    start=(kb == 0),
                                    stop=(kb == S // 128 - 1),
                                )
                            nc.vector.tensor_copy(
                                attnT[dlo:dlo + 64, p,
                                      pan * PANEL + half * 256:
                                      pan * PANEL + (half + 1) * 256],
                                pspv,
                            )

            # ---------------- Phase 3: out projection (partial) ----------------
            with tc.tile_pool(name="ph3", bufs=3) as ph3:
                for g in range(S // 512):
                    ot = ph3.tile([128, 4, DM], F32, tag="ot")
                    for s4 in range(4):
                        sc = g * 4 + s4
                        for nh in range(2):
                            pso = ps_big.tile([128, 512], F32, tag="sc")
                            for p in range(NPAIR):
                                nc.tensor.matmul(
                                    pso,
                                    attnT[:, p, sc * 128:(sc + 1) * 128],
                                    wo_sb[:, p, nh * 512:(nh + 1) * 512],
                                    start=(p == 0),
                                    stop=(p == NPAIR - 1),
                                )
                            if (sc + nh) % 2 == 0:
                                nc.scalar.copy(
                                    ot[:, s4, nh * 512:(nh + 1) * 512], pso
                                )
                            else:
                                nc.vector.tensor_copy(
                                    ot[:, s4, nh * 512:(nh + 1) * 512], pso
                                )
                    nc.sync.dma_start(
                        out=y_out[g * 512:(g + 1) * 512, :].rearrange(
                            "(s4 pp) d -> pp s4 d", pp=128
                        ),
                        in_=ot,
                    )

    nc.compile()
    return nc


_NC_CACHE = None


def _get_nc():
    global _NC_CACHE
    if _NC_CACHE is None:
        _NC_CACHE = build_nc()
    return _NC_CACHE


def make_in_maps(query, key, value, Wq, bq, Wk, bk, Wv, bv, Wo, bo):
    query = np.asarray(query, np.float32)
    key = np.asarray(key, np.float32)
    value = np.asarray(value, np.float32)
    # host-side layout prep: transpose activations once per batch
    xqt = [np.ascontiguousarray(query[b].T) for b in range(B)]
    xkt = [np.ascontiguousarray(key[b].T) for b in range(B)]
    xvt = [np.ascontiguousarray(value[b].T) for b in range(B)]
    in_maps = []
    for c in range(NCORES):
        b = c // 4
        hp = c % 4
        sl = slice(hp * DPC, (hp + 1) * DPC)
        in_maps.append({
            "xqt": xqt[b],
            "xkt": xkt[b],
            "xvt": xvt[b],
            "wq": np.ascontiguousarray(np.asarray(Wq, np.float32)[:, sl]),
            "wk": np.ascontiguousarray(np.asarray(Wk, np.float32)[:, sl]),
            "wv": np.ascontiguousarray(np.asarray(Wv, np.float32)[:, sl]),
            "wo": np.ascontiguousarray(np.asarray(Wo, np.float32)[sl, :]),
            "bq": np.ascontiguousarray(np.asarray(bq, np.float32)[sl]),
            "bk": np.ascontiguousarray(np.asarray(bk, np.float32)[sl]),
            "bv": np.ascontiguousarray(np.asarray(bv, np.float32)[sl]),
        })
    return in_maps


def gather_outputs(results, bo):
    probs = np.empty((B, H, S, S), np.float32)
    out = np.zeros((B, S, DM), np.float32)
    for c in range(NCORES):
        b = c // 4
        hp = c % 4
        probs[b, hp * HPC:(hp + 1) * HPC] = results[c]["probs"]
        out[b] += results[c]["y"]
    out += np.asarray(bo, np.float32)
    return out, probs


def kernel(query, key, value, Wq, bq, Wk, bk, Wv, bv, Wo, bo):
    nc = _get_nc()
    in_maps = make_in_maps(query, key, value, Wq, bq, Wk, bk, Wv, bv, Wo, bo)
    res = run_bass_kernel_spmd(nc, in_maps, core_ids=list(range(NCORES)))
    return gather_outputs(res.results, bo)


# revision 61
# speedup vs baseline: 1.0051x; 1.0051x over previous
"""Multi-head attention (B=2, S=2048, D=1024, H=16) on 8 trn2 NeuronCores.

Sharding: data-parallel over batch (cores 0-3 -> b=0, cores 4-7 -> b=1),
tensor-parallel over heads within each batch group (4 heads per core).
Each core:
  - projects q/k/v with its 256-column slice of Wq/Wk/Wv (heads h0..h0+3),
  - computes scores/softmax/PV for its 4 heads (emitting the probs slice),
  - computes a partial out-projection with its 256-row slice of Wo.
Host passes activations pre-transposed to [d_model, s] (layout prep, like
the weight slices), gathers probs slices, and sums the partial outputs.

Projection order v, k, q: the last projection before attention is q, whose
first s-chunk unblocks head 0's first panel, so the probs DMA stream starts
while later q chunks are still loading.

All matmuls run as float32r (full PE rate at N>=256, ~1e-4 rel err).
PSUM pools are shared across stages via common tags (no pool barriers).
"""

import numpy as np

import concourse.bass as bass
import concourse.mybir as mybir
import concourse.tile as tile
from concourse import bacc
from concourse.bass_utils import run_bass_kernel_spmd
from concourse.masks import make_identity

B = 2
S = 2048
DM = 1024
H = 16
DK = 64
NCORES = 8
HPC = 4            # heads per core
DPC = HPC * DK     # 256: d_out slice per core
NPAIR = HPC // 2   # 2 head pairs per core
F32 = mybir.dt.float32
F32R = mybir.dt.float32r

PANEL = 512        # q rows per phase-2 panel
NPANEL = S // PANEL
QSUB = 128         # q rows per scores matmul
NQSUB = PANEL // QSUB
KC = 512           # k columns per scores matmul
NKC = S // KC
SC = 512           # s columns per phase-1 chunk
NSC = S // SC
NKB = DM // 128    # d_model k-chunks for projections


def build_nc():
    nc = bacc.Bacc("TRN2", target_bir_lowering=False)

    xqt = nc.dram_tensor("xqt", [DM, S], F32R, kind="ExternalInput")
    xkt = nc.dram_tensor("xkt", [DM, S], F32R, kind="ExternalInput")
    xvt = nc.dram_tensor("xvt", [DM, S], F32R, kind="ExternalInput")
    wq = nc.dram_tensor("wq", [DM, DPC], F32R, kind="ExternalInput")
    wk = nc.dram_tensor("wk", [DM, DPC], F32R, kind="ExternalInput")
    wv = nc.dram_tensor("wv", [DM, DPC], F32R, kind="ExternalInput")
    wo = nc.dram_tensor("wo", [DPC, DM], F32R, kind="ExternalInput")
    bq = nc.dram_tensor("bq", [DPC], F32, kind="ExternalInput")
    bk = nc.dram_tensor("bk", [DPC], F32, kind="ExternalInput")
    bv = nc.dram_tensor("bv", [DPC], F32, kind="ExternalInput")

    probs_out = nc.dram_tensor("probs", [HPC, S, S], F32R, kind="ExternalOutput")
    y_out = nc.dram_tensor("y", [S, DM], F32, kind="ExternalOutput")

    with tile.TileContext(nc) as tc:
        with (
            tc.tile_pool(name="persist", bufs=1) as persist,
            tc.tile_pool(name="ps_big", bufs=2, space="PSUM") as ps_big,
            tc.tile_pool(name="ps_mid", bufs=2, space="PSUM") as ps_mid,
            tc.tile_pool(name="ps_sml", bufs=2, space="PSUM") as ps_sml,
        ):
            ident_f = persist.tile([128, 128], F32)
            make_identity(nc, ident_f)
            ident = persist.tile([128, 128], F32R)
            nc.vector.tensor_copy(ident, ident_f)

            qT = persist.tile([128, NPAIR, S], F32R)
            kT = persist.tile([128, NPAIR, S], F32R)
            vN = persist.tile([128, S // 128, DPC], F32R)
            attnT = persist.tile([128, NPAIR, S], F32R)
            wo_sb = persist.tile([128, NPAIR, DM], F32R)

            # ---------------- Phase 1: projections (v, k, q) ----------------
            with (
                tc.tile_pool(name="ph1w", bufs=1) as ph1w,
                tc.tile_pool(name="ph1", bufs=3) as ph1,
            ):
                xTv0 = ph1.tile([128, NKB, SC], F32R, tag="xT")
                nc.sync.dma_start(
                    out=xTv0,
                    in_=xvt[:, 0:SC].rearrange("(kb p) s -> p kb s", p=128),
                )
                wv_sb = ph1w.tile([128, NKB, DPC], F32R)
                wk_sb = ph1w.tile([128, NKB, DPC], F32R)
                wq_sb = ph1w.tile([128, NKB, DPC], F32R)
                nc.sync.dma_start(
                    out=wv_sb, in_=wv[:].rearrange("(kc p) n -> p kc n", p=128)
                )
                nc.sync.dma_start(
                    out=wk_sb, in_=wk[:].rearrange("(kc p) n -> p kc n", p=128)
                )
                nc.sync.dma_start(
                    out=wq_sb, in_=wq[:].rearrange("(kc p) n -> p kc n", p=128)
                )
                bq_sb = ph1w.tile([128, 2], F32)
                bk_sb = ph1w.tile([128, 2], F32)
                nc.sync.dma_start(out=bq_sb, in_=bq[:].rearrange("(m p) -> p m", p=128))
                nc.sync.dma_start(out=bk_sb, in_=bk[:].rearrange("(m p) -> p m", p=128))
                bv_sb = ph1w.tile([128, DPC], F32)
                bv_ap = bv[:]
                nc.sync.dma_start(
                    out=bv_sb,
                    in_=bass.AP(
                        tensor=bv_ap.tensor, offset=bv_ap.offset,
                        ap=[[0, 128], [1, DPC]],
                    ),
                )

                for which, xinT in (("v", xvt), ("k", xkt), ("q", xqt)):
                    for sc in range(NSC):
                        if which == "v" and sc == 0:
                            xTt = xTv0
                        else:
                            xTt = ph1.tile([128, NKB, SC], F32R, tag="xT")
                            nc.sync.dma_start(
                                out=xTt,
                                in_=xinT[:, sc * SC:(sc + 1) * SC].rearrange(
                                    "(kb p) s -> p kb s", p=128
                                ),
                            )
                        if which == "v":
                            for ss in range(SC // 128):
                                psv = ps_sml.tile([128, DPC], F32, tag="s")
                                for kb in range(NKB):
                                    nc.tensor.matmul(
                                        psv,
                                        xTt[:, kb, ss * 128:(ss + 1) * 128],
                                        wv_sb[:, kb, :],
                                        start=(kb == 0),
                                        stop=(kb == NKB - 1),
                                    )
                                nc.vector.tensor_add(
                                    vN[:, sc * (SC // 128) + ss, :], psv, bv_sb
                                )
                        else:
                            dst = qT if which == "q" else kT
                            bias = bq_sb if which == "q" else bk_sb
                            w_sb = wq_sb if which == "q" else wk_sb
                            for m in range(NPAIR):
                                psq = ps_mid.tile([128, SC], F32, tag="m")
                                for kb in range(NKB):
                                    nc.tensor.matmul(
                                        psq,
                                        w_sb[:, kb, m * 128:(m + 1) * 128],
                                        xTt[:, kb, :],
                                        start=(kb == 0),
                                        stop=(kb == NKB - 1),
                                    )
                                nc.vector.tensor_scalar_add(
                                    dst[:, m, sc * SC:(sc + 1) * SC],
                                    psq,
                                    bias[:, m:m + 1],
                                )

            # ---------------- Phase 2: attention per head ----------------
            with tc.tile_pool(name="ph2", bufs=1) as ph2:
                nc.sync.dma_start(
                    out=wo_sb, in_=wo[:].rearrange("(m p) n -> p m n", p=128)
                )
                for h in range(HPC):
                    p, hh = h // 2, h % 2
                    dlo = 64 * hh
                    for pan in range(NPANEL):
                        for half in range(NQSUB // 2):
                            pT = ph2.tile([128, S // 128, PANEL // 2], F32R,
                                          tag="pT", bufs=2)
                            eh = ph2.tile([128, 2, S], F32R, tag="exp", bufs=4)
                            for q2 in range(2):
                                qs = half * 2 + q2
                                q0 = pan * PANEL + qs * QSUB
                                den = ph2.tile([128, 2], F32, tag="den", bufs=8)
                                for kh in range(2):
                                    ps = ps_big.tile([128, 2 * KC], F32, tag="sc")
                                    for k2 in range(2):
                                        kc = 2 * kh + k2
                                        nc.tensor.matmul(
                                            ps[:, k2 * KC:(k2 + 1) * KC],
                                            qT[dlo:dlo + 64, p, q0:q0 + QSUB],
                                            kT[dlo:dlo + 64, p,
                                               kc * KC:(kc + 1) * KC],
                                            start=True, stop=True,
                                        )
                                    nc.scalar.activation(
                                        out=eh[:, q2, kh * 1024:(kh + 1) * 1024],
                                        in_=ps,
                                        func=mybir.ActivationFunctionType.Exp,
                                        scale=0.125,
                                        accum_out=den[:, kh:kh + 1],
                                    )
                                rcp = ph2.tile([128, 1], F32, tag="rcp", bufs=8)
                                nc.vector.reduce_sum(
                                    out=rcp, in_=den, axis=mybir.AxisListType.X
                                )
                                nc.vector.reciprocal(out=rcp, in_=rcp)
                                nc.vector.tensor_scalar_mul(
                                    eh[:, q2, :], eh[:, q2, :], rcp
                                )
                                for kbg in range(4):
                                    pst2 = ps_mid.tile([128, 4, 128], F32R, tag="m")
                                    for k4 in range(4):
                                        kb = kbg * 4 + k4
                                        nc.tensor.transpose(
                                            pst2[:, k4, :],
                                            eh[:, q2, kb * 128:(kb + 1) * 128],
                                            ident,
                                        )
                                    dst = pT[:, kbg * 4:(kbg + 1) * 4,
                                             q2 * 128:(q2 + 1) * 128]
                                    if (q2 * 4 + kbg) % 8 < 5:
                                        nc.vector.tensor_copy(dst, pst2)
                                    else:
                                        nc.scalar.copy(dst, pst2)
                            q0 = pan * PANEL + half * 2 * QSUB
                            nc.sync.dma_start(
                                out=probs_out[h, q0:q0 + 2 * QSUB, :].rearrange(
                                    "(q2 pp) k -> pp q2 k", pp=128
                                ),
                                in_=eh,
                            )
                            pspv = ps_sml.tile([64, PANEL // 2], F32, tag="s")
                            lo = h * DK
                            for kb in range(S // 128):
                                nc.tensor.matmul(
                                    pspv,
                                    vN[:, kb, lo:lo + DK],
                                    pT[:, kb, :],
                                    start=(kb == 0),
                                    stop=(kb == S // 128 - 1),
                                )
                            nc.vector.tensor_copy(
                                attnT[dlo:dlo + 64, p,
                                      pan * PANEL + half * 256:
                                      pan * PANEL + (half + 1) * 256],
                                pspv,
                            )

            # ---------------- Phase 3: out projection (partial) ----------------
            with tc.tile_pool(name="ph3", bufs=3) as ph3:
                for g in range(S // 512):
                    ot = ph3.tile([128, 4, DM], F32, tag="ot")
                    for s4 in range(4):
                        sc = g * 4 + s4
                        for nh in range(2):
                            if (sc * 2 + nh) % 2 == 0:
                                pso = ps_big.tile([128, 512], F32, tag="sc")
                            else:
                                pso = ps_mid.tile([128, 512], F32, tag="m")
                            for p in range(NPAIR):
                                nc.tensor.matmul(
                                    pso,
                                    attnT[:, p, sc * 128:(sc + 1) * 128],
                                    wo_sb[:, p, nh * 512:(nh + 1) * 512],
                                    start=(p == 0),
                                    stop=(p == NPAIR - 1),
                                )
                            if (sc + nh) % 2 == 0:
                                nc.scalar.copy(
                                    ot[:, s4, nh * 512:(nh + 1) * 512], pso
                                )
                            else:
                                nc.vector.tensor_copy(
                                    ot[:, s4, nh * 512:(nh + 1) * 512], pso
                                )
                    nc.sync.dma_start(
                        out=y_out[g * 512:(g + 1) * 512, :].rearrange(
                            "(s4 pp) d -> pp s4 d", pp=128
                        ),
                        in_=ot,
                    )

    nc.compile()
    return nc


_NC_CACHE = None


def _get_nc():
    global _NC_CACHE
    if _NC_CACHE is None:
        _NC_CACHE = build_nc()
    return _NC_CACHE


def make_in_maps(query, key, value, Wq, bq, Wk, bk, Wv, bv, Wo, bo):
    query = np.asarray(query, np.float32)
    key = np.asarray(key, np.float32)
    value = np.asarray(value, np.float32)
    # host-side layout prep: transpose activations once per batch
    xqt = [np.ascontiguousarray(query[b].T) for b in range(B)]
    xkt = [np.ascontiguousarray(key[b].T) for b in range(B)]
    xvt = [np.ascontiguousarray(value[b].T) for b in range(B)]
    in_maps = []
    for c in range(NCORES):
        b = c // 4
        hp = c % 4
        sl = slice(hp * DPC, (hp + 1) * DPC)
        in_maps.append({
            "xqt": xqt[b],
            "xkt": xkt[b],
            "xvt": xvt[b],
            "wq": np.ascontiguousarray(np.asarray(Wq, np.float32)[:, sl]),
            "wk": np.ascontiguousarray(np.asarray(Wk, np.float32)[:, sl]),
            "wv": np.ascontiguousarray(np.asarray(Wv, np.float32)[:, sl]),
            "wo": np.ascontiguousarray(np.asarray(Wo, np.float32)[sl, :]),
            "bq": np.ascontiguousarray(np.asarray(bq, np.float32)[sl]),
            "bk": np.ascontiguousarray(np.asarray(bk, np.float32)[sl]),
            "bv": np.ascontiguousarray(np.asarray(bv, np.float32)[sl]),
        })
    return in_maps


def gather_outputs(results, bo):
    probs = np.empty((B, H, S, S), np.float32)
    out = np.zeros((B, S, DM), np.float32)
    for c in range(NCORES):
        b = c // 4
        hp = c % 4
        probs[b, hp * HPC:(hp + 1) * HPC] = results[c]["probs"]
        out[b] += results[c]["y"]
    out += np.asarray(bo, np.float32)
    return out, probs


def kernel(query, key, value, Wq, bq, Wk, bk, Wv, bv, Wo, bo):
    nc = _get_nc()
    in_maps = make_in_maps(query, key, value, Wq, bq, Wk, bk, Wv, bv, Wo, bo)
    res = run_bass_kernel_spmd(nc, in_maps, core_ids=list(range(NCORES)))
    return gather_outputs(res.results, bo)
